# revision 1
# baseline (speedup 1.0000x reference)
"""Distributed single-head causal attention for Trainium2 (8 NeuronCores).

Problem: x:[4,2048,1024] f32, Wq/Wk/Wv/Wo:[1024,1024], b*:[1024]
  q = x@Wq.T+bq; k = x@Wk.T+bk; v = x@Wv.T+bv
  scores = (q@k.T)/sqrt(1024) with causal mask; attn = softmax
  out = (attn@v)@Wo.T + bo

Sharding (data-parallel, causal-balanced, strict-SPMD):
  8 cores = 4 batches x 2 cores/batch. The 16 query blocks (128 rows) of a
  batch are split by parity: core (b, even-core) takes odd blocks
  {1,3,..,15}, core (b, odd-core) takes even blocks {0,2,..,14}. Both cores
  process 8 "slots" with the SAME compile-time key-range schedule
  T_s = 256*(s+1), so the instruction stream is identical on all cores
  (required: one NEFF, SPMD) and causal work is perfectly balanced.
  Host gathers each core's query rows into a contiguous buffer and prepares
  a per-core additive mask for the 256 diagonal columns of each slot.

Compute (per core, all matmuls bf16, fp32 PSUM accumulate):
  QT[e,s]   = WqT.T @ xqT   (scaled by 1/32, +bq/32)   feature-major
  KT[e,t]   = WkT.T @ xkvT  (+bk)                      feature-major
  V[t,d]    = xkvT.T @ WvT  (+bv)                      natural
  scores[s,t] = QT.T @ KT   per slot, psum; +mask; softmax via
                reduce_max(negate) + Exp(bias=-max, accum_out=l)
  attnT[t,s] via DMA-transpose (bf16 xbar, off the PE)
  ctxT[d,s] = V.T-tiles @ attnT  (causal-ragged suffix accumulation)
  out[s,e]  = ctxT.T @ WoT, *1/l (per-partition scale), +bo
"""

import sys

if "/opt/trn_rl_repo" not in sys.path:
    sys.path.insert(0, "/opt/trn_rl_repo")

import numpy as np
import ml_dtypes

import concourse.bass as bass
import concourse.mybir as mybir
from concourse import bacc
from concourse.bass_utils import run_bass_kernel_spmd
from concourse.tile import TileContext

B, S, D = 4, 2048, 1024
NB = S // 128          # 16 query blocks per batch
NSLOT = 8              # slots per core
EC = D // 128          # 8 feature chunks
F32 = mybir.dt.float32
BF16 = mybir.dt.bfloat16
NEG = -1.0e9

_compiled = None  # (nc, out_name)


def _slot_T(s):
    return 256 * (s + 1)


def _build():
    nc = bacc.Bacc("TRN2", target_bir_lowering=False, debug=False, num_devices=8)

    xqT = nc.dram_tensor("xqT", [128, EC, 1024], BF16, kind="ExternalInput")
    xkvT = nc.dram_tensor("xkvT", [128, EC, S], BF16, kind="ExternalInput")
    wqT = nc.dram_tensor("wqT", [128, EC, D], BF16, kind="ExternalInput")
    wkT = nc.dram_tensor("wkT", [128, EC, D], BF16, kind="ExternalInput")
    wvT = nc.dram_tensor("wvT", [128, EC, D], BF16, kind="ExternalInput")
    woT = nc.dram_tensor("woT", [128, EC, D], BF16, kind="ExternalInput")
    bq_d = nc.dram_tensor("bq", [128, EC], F32, kind="ExternalInput")
    bk_d = nc.dram_tensor("bk", [128, EC], F32, kind="ExternalInput")
    bv_d = nc.dram_tensor("bv", [1, D], F32, kind="ExternalInput")
    bo_d = nc.dram_tensor("bo", [1, D], F32, kind="ExternalInput")
    mask_d = nc.dram_tensor("mask", [128, NSLOT, 256], F32, kind="ExternalInput")
    out_d = nc.dram_tensor("out", [NSLOT * 128, D], F32, kind="ExternalOutput")

    inv = 1.0 / 32.0

    with TileContext(nc) as tc:
        # ---- persistent activations ----
        with (
            tc.tile_pool(name="persist", bufs=1) as persist,
            tc.tile_pool(name="small", bufs=1) as small,
        ):
            QT = persist.tile([128, EC, 1024], BF16, tag="QT")
            KT = persist.tile([128, EC, S], BF16, tag="KT")
            V = persist.tile([128, NB, D], BF16, tag="V")
            MASK = small.tile([128, NSLOT, 256], F32, tag="MASK")
            BQ = small.tile([128, EC], F32, tag="BQ")
            BK = small.tile([128, EC], F32, tag="BK")
            RL = small.tile([128, NSLOT], F32, tag="RL")
            BOF = small.tile([128, D], F32, tag="BOF")

            nc.sync.dma_start(out=MASK[:, :, :], in_=mask_d[:, :, :])
            nc.sync.dma_start(out=BK[:, :], in_=bk_d[:, :])
            bq_raw = small.tile([128, EC], F32, tag="bq_raw")
            nc.sync.dma_start(out=bq_raw[:, :], in_=bq_d[:, :])
            nc.scalar.mul(BQ[:, :], bq_raw[:, :], inv)
            bo_row = small.tile([1, D], F32, tag="bo_row")
            nc.sync.dma_start(out=bo_row[:, :], in_=bo_d[:, :])
            nc.gpsimd.partition_broadcast(BOF[:, :], bo_row[:1, :])

            # ---- phase A: projections ----
            with (
                tc.tile_pool(name="xin", bufs=1) as xin,
                tc.tile_pool(name="wts", bufs=1) as wts,
                tc.tile_pool(name="pa_psum", bufs=4, space="PSUM") as pa_psum,
            ):
                XQ = xin.tile([128, EC, 1024], BF16, tag="XQ")
                XK = xin.tile([128, EC, S], BF16, tag="XK")
                WQ = wts.tile([128, EC, D], BF16, tag="WQ")
                WK = wts.tile([128, EC, D], BF16, tag="WK")
                WV = wts.tile([128, EC, D], BF16, tag="WV")
                BVF = xin.tile([128, D], F32, tag="BVF")
                bv_row = small.tile([1, D], F32, tag="bv_row")
                nc.sync.dma_start(out=bv_row[:, :], in_=bv_d[:, :])
                nc.gpsimd.partition_broadcast(BVF[:, :], bv_row[:1, :])

                nc.sync.dma_start(out=XQ[:, :, :], in_=xqT[:, :, :])
                nc.sync.dma_start(out=XK[:, :, :], in_=xkvT[:, :, :])
                nc.sync.dma_start(out=WQ[:, :, :], in_=wqT[:, :, :])
                nc.sync.dma_start(out=WK[:, :, :], in_=wkT[:, :, :])
                nc.sync.dma_start(out=WV[:, :, :], in_=wvT[:, :, :])

                # QT[e, s] (pre-scaled by 1/32): lhsT = WQ[d,e], rhs = XQ[d,s]
                for ec in range(EC):
                    for sh in range(2):
                        ps = pa_psum.tile([128, 512], F32, tag="pa")
                        for dc in range(EC):
                            nc.tensor.matmul(
                                ps[:, :],
                                WQ[:, dc, ec * 128 : (ec + 1) * 128],
                                XQ[:, dc, sh * 512 : (sh + 1) * 512],
                                start=(dc == 0),
                                stop=(dc == EC - 1),
                            )
                        nc.scalar.activation(
                            QT[:, ec, sh * 512 : (sh + 1) * 512],
                            ps[:, :],
                            mybir.ActivationFunctionType.Identity,
                            bias=BQ[:, ec : ec + 1],
                            scale=inv,
                        )

                # KT[e, t]: lhsT = WK[d,e], rhs = XK[d,t]
                for ec in range(EC):
                    for th in range(S // 512):
                        ps = pa_psum.tile([128, 512], F32, tag="pa")
                        for dc in range(EC):
                            nc.tensor.matmul(
                                ps[:, :],
                                WK[:, dc, ec * 128 : (ec + 1) * 128],
                                XK[:, dc, th * 512 : (th + 1) * 512],
                                start=(dc == 0),
                                stop=(dc == EC - 1),
                            )
                        nc.scalar.activation(
                            KT[:, ec, th * 512 : (th + 1) * 512],
                            ps[:, :],
                            mybir.ActivationFunctionType.Identity,
                            bias=BK[:, ec : ec + 1],
                            scale=1.0,
                        )

                # V[t, d] natural: lhsT = XK[d, t-block], rhs = WV[d, e]
                for tb in range(NB):
                    for dh in range(2):
                        ps = pa_psum.tile([128, 512], F32, tag="pa")
                        for dc in range(EC):
                            nc.tensor.matmul(
                                ps[:, :],
                                XK[:, dc, tb * 128 : (tb + 1) * 128],
                                WV[:, dc, dh * 512 : (dh + 1) * 512],
                                start=(dc == 0),
                                stop=(dc == EC - 1),
                            )
                        nc.vector.tensor_tensor(
                            out=V[:, tb, dh * 512 : (dh + 1) * 512],
                            in0=ps[:, :],
                            in1=BVF[:, dh * 512 : (dh + 1) * 512],
                            op=mybir.AluOpType.add,
                        )

            # ---- phase B: attention ----
            with (
                tc.tile_pool(name="wo", bufs=1) as wo_pool,
                tc.tile_pool(name="att", bufs=5) as att_pool,
                tc.tile_pool(name="attT", bufs=2) as attT_pool,
                tc.tile_pool(name="ctx", bufs=1) as ctx_pool,
                tc.tile_pool(name="stat", bufs=1) as stat_pool,
                tc.tile_pool(name="sc_psum", bufs=3, space="PSUM") as sc_psum,
                tc.tile_pool(name="mm_psum", bufs=2, space="PSUM") as mm_psum,
                tc.tile_pool(name="outbuf", bufs=2) as out_pool,
            ):
                WO = wo_pool.tile([128, EC, D], BF16, tag="WO")
                nc.sync.dma_start(out=WO[:, :, :], in_=woT[:, :, :])
                CTXT = ctx_pool.tile([128, EC, 1024], BF16, tag="CTXT")
                NM = stat_pool.tile([128, 2], F32, tag="NM")
                LSUM = stat_pool.tile([128, 2], F32, tag="LS")
                LTOT = stat_pool.tile([128, 1], F32, tag="LT")

                att_tiles = {}
                attT_tiles = {}

                for g in range(2):  # two groups of 4 slots
                    ATT_T = attT_pool.tile([128, NB, 512], BF16, tag="attT")
                    attT_tiles[g] = ATT_T
                    for j in range(4):
                        slot = g * 4 + j
                        T = _slot_T(slot)
                        nt = T // 128
                        ATT = att_pool.tile([128, S], BF16, tag="att")
                        att_tiles[slot] = ATT

                        # scores psum tiles of up to 1024 cols
                        nparts = (T + 1023) // 1024
                        parts = []
                        for p in range(nparts):
                            w = min(1024, T - p * 1024)
                            sc = sc_psum.tile([128, 1024], F32, tag="sc")
                            parts.append((sc, w))
                            for c0 in range(0, w, 512):
                                cw = min(512, w - c0)
                                for ec in range(EC):
                                    nc.tensor.matmul(
                                        sc[:, c0 : c0 + cw],
                                        QT[:, ec, slot * 128 : (slot + 1) * 128],
                                        KT[:, ec, p * 1024 + c0 : p * 1024 + c0 + cw],
                                        start=(ec == 0),
                                        stop=(ec == EC - 1),
                                    )
                        # additive causal mask on the last 256 cols
                        lsc, lw = parts[-1]
                        nc.vector.tensor_tensor(
                            out=lsc[:, lw - 256 : lw],
                            in0=lsc[:, lw - 256 : lw],
                            in1=MASK[:, slot, :],
                            op=mybir.AluOpType.add,
                        )
                        # -max per part, combined
                        for p, (sc, w) in enumerate(parts):
                            nc.vector.reduce_max(
                                out=NM[:, p : p + 1],
                                in_=sc[:, :w],
                                axis=mybir.AxisListType.X,
                                negate=True,
                            )
                        if nparts == 2:
                            nc.vector.tensor_tensor(
                                out=NM[:, 0:1],
                                in0=NM[:, 0:1],
                                in1=NM[:, 1:2],
                                op=mybir.AluOpType.min,
                            )
                        # exp with accumulated row sum
                        for p, (sc, w) in enumerate(parts):
                            nc.scalar.activation(
                                ATT[:, p * 1024 : p * 1024 + w],
                                sc[:, :w],
                                mybir.ActivationFunctionType.Exp,
                                bias=NM[:, 0:1],
                                scale=1.0,
                                accum_out=LSUM[:, p : p + 1],
                            )
                        if nparts == 2:
                            nc.vector.tensor_tensor(
                                out=LTOT[:, 0:1],
                                in0=LSUM[:, 0:1],
                                in1=LSUM[:, 1:2],
                                op=mybir.AluOpType.add,
                            )
                            nc.vector.reciprocal(RL[:, slot : slot + 1], LTOT[:, 0:1])
                        else:
                            nc.vector.reciprocal(RL[:, slot : slot + 1], LSUM[:, 0:1])

                        # transpose attn -> [t, s-in-group] tiles (off-PE, xbar)
                        for tcn in range(nt):
                            nc.sync.dma_start_transpose(
                                ATT_T[:, tcn, j * 128 : (j + 1) * 128],
                                ATT[:, tcn * 128 : (tcn + 1) * 128],
                            )

                    # group AV: ctxT[d, s] accumulated over t chunks
                    ntg = _slot_T(g * 4 + 3) // 128  # 8 or 16
                    for dc in range(EC):
                        ps = mm_psum.tile([128, 512], F32, tag="mm")
                        for tcn in range(ntg):
                            jmin = max(0, (tcn + 1 - 1) // 2 - g * 4)
                            # slot j in group valid iff T=256(g*4+j+1) >= 128(tcn+1)
                            jmin = 0
                            for jj in range(4):
                                if 256 * (g * 4 + jj + 1) >= 128 * (tcn + 1):
                                    jmin = jj
                                    break
                            scol = jmin * 128
                            nc.tensor.matmul(
                                ps[:, scol:512],
                                V[:, tcn, dc * 128 : (dc + 1) * 128],
                                attT_tiles[g][:, tcn, scol:512],
                                start=(tcn == 0),
                                stop=(tcn == ntg - 1),
                            )
                        nc.vector.tensor_copy(
                            CTXT[:, dc, g * 512 : (g + 1) * 512], ps[:, :]
                        )

                # ---- phase C: output projection ----
                for slot in range(NSLOT):
                    OUTS = out_pool.tile([128, D], F32, tag="outs")
                    for eh in range(2):
                        ps = mm_psum.tile([128, 512], F32, tag="mm")
                        for dc in range(EC):
                            nc.tensor.matmul(
                                ps[:, :],
                                CTXT[:, dc, slot * 128 : (slot + 1) * 128],
                                WO[:, dc, eh * 512 : (eh + 1) * 512],
                                start=(dc == 0),
                                stop=(dc == EC - 1),
                            )
                        # out = psum * (1/l) + bo
                        nc.vector.scalar_tensor_tensor(
                            out=OUTS[:, eh * 512 : (eh + 1) * 512],
                            in0=ps[:, :],
                            scalar=RL[:, slot : slot + 1],
                            in1=BOF[:, eh * 512 : (eh + 1) * 512],
                            op0=mybir.AluOpType.mult,
                            op1=mybir.AluOpType.add,
                        )
                    nc.sync.dma_start(
                        out=out_d[slot * 128 : (slot + 1) * 128, :], in_=OUTS[:, :]
                    )

    nc.compile()
    return nc


def _core_blocks(core):
    """Absolute 128-row query block index per slot for this core."""
    parity = 1 if core % 2 == 0 else 0  # even core -> odd blocks
    return [2 * s + parity for s in range(NSLOT)]


def _make_in_maps(x, Wq, bq, Wk, bk, Wv, bv, Wo, bo):
    bf = ml_dtypes.bfloat16

    def wt_layout(W):
        # W:[e,d] -> W.T:[d,e] -> [128 part, EC chunk, e]
        return np.ascontiguousarray(
            W.T.astype(bf).reshape(EC, 128, D).transpose(1, 0, 2)
        )

    wq_l, wk_l, wv_l, wo_l = (wt_layout(W) for W in (Wq, Wk, Wv, Wo))
    bq_l = np.ascontiguousarray(bq.reshape(EC, 128).T.astype(np.float32))
    bk_l = np.ascontiguousarray(bk.reshape(EC, 128).T.astype(np.float32))
    bv_l = np.ascontiguousarray(bv.reshape(1, D).astype(np.float32))
    bo_l = np.ascontiguousarray(bo.reshape(1, D).astype(np.float32))

    in_maps = []
    for core in range(8):
        b = core // 2
        blocks = _core_blocks(core)
        xb = np.asarray(x[b], dtype=np.float32)  # [S, D]
        xq = np.concatenate([xb[bl * 128 : (bl + 1) * 128] for bl in blocks], axis=0)
        xqT_l = np.ascontiguousarray(
            xq.T.astype(bf).reshape(EC, 128, 1024).transpose(1, 0, 2)
        )
        xkvT_l = np.ascontiguousarray(
            xb.T.astype(bf).reshape(EC, 128, S).transpose(1, 0, 2)
        )
        # mask[r, slot, j]: key t = 256*slot + j valid iff t <= 128*block + r
        mask = np.zeros((128, NSLOT, 256), np.float32)
        r = np.arange(128)[:, None]
        jj = np.arange(256)[None, :]
        for s_i, bl in enumerate(blocks):
            lim = bl * 128 + r  # [128,1]
            t_idx = 256 * s_i + jj
            mask[:, s_i, :] = np.where(t_idx <= lim, 0.0, NEG)
        in_maps.append(
            {
                "xqT": xqT_l,
                "xkvT": xkvT_l,
                "wqT": wq_l,
                "wkT": wk_l,
                "wvT": wv_l,
                "woT": wo_l,
                "bq": bq_l,
                "bk": bk_l,
                "bv": bv_l,
                "bo": bo_l,
                "mask": mask,
            }
        )
    return in_maps


def _run(inputs, trace=False):
    global _compiled
    if _compiled is None:
        _compiled = _build()
    nc = _compiled
    in_maps = _make_in_maps(**inputs)
    res = run_bass_kernel_spmd(nc, in_maps, core_ids=list(range(8)), trace=trace)
    out = np.zeros((B, S, D), np.float32)
    for core in range(8):
        b = core // 2
        o = res.results[core]["out"]  # [1024, D] slot-major
        for s_i, bl in enumerate(_core_blocks(core)):
            out[b, bl * 128 : (bl + 1) * 128, :] = o[s_i * 128 : (s_i + 1) * 128, :]
    return out, res


def kernel(**inputs):
    out, _ = _run(inputs, trace=False)
    return out


# revision 2
# speedup vs baseline: 1.3784x; 1.3784x over previous
"""Distributed single-head causal attention for Trainium2 (8 NeuronCores).

Problem: x:[4,2048,1024] f32, Wq/Wk/Wv/Wo:[1024,1024], b*:[1024]
  q = x@Wq.T+bq; k = x@Wk.T+bk; v = x@Wv.T+bv
  scores = (q@k.T)/sqrt(1024) causal-masked; out = softmax(scores)@v @Wo.T + bo

Sharding (data-parallel, causal-balanced, strict-SPMD):
  8 cores = 4 batches x 2 cores/batch. The 16 query blocks (128 rows) of a
  batch are split by parity (odd blocks / even blocks). Both cores process 8
  "slots" with the same compile-time key-range schedule T_s = 256*(s+1), so
  the instruction stream is identical on all cores (one NEFF, SPMD) and
  causal work is perfectly balanced. Host gathers each core's query rows
  into a contiguous buffer and prepares a per-core additive mask for the
  256 diagonal columns of each slot.

Compute (per core, bf16 matmuls, fp32 PSUM):
  QT[e,s] = WqT.T @ xqT (x1/32, +bq/32); KT[e,t] = WkT.T @ xkvT (+bk)
  V[t,d] = xkvT.T @ WvT (+bv)
  per slot: scores = QT.T @ KT in psum; +mask; softmax (reduce_max negate ->
  Exp bias=-max accum_out=l); attnT via one batched bf16 xbar DMA-transpose;
  ctxT[d,s] = V-tiles.T @ attnT (ragged suffix accumulation over t)
  out[s,e] = ctxT.T @ WoT, * (1/l) + bo  (fused DVE scalar_tensor_tensor)
"""

import sys

if "/opt/trn_rl_repo" not in sys.path:
    sys.path.insert(0, "/opt/trn_rl_repo")

import numpy as np
import ml_dtypes

import concourse.bass as bass
import concourse.mybir as mybir
from concourse import bacc
from concourse.bass_utils import run_bass_kernel_spmd
from concourse.tile import TileContext

B, S, D = 4, 2048, 1024
NB = S // 128          # 16 key blocks per batch
NSLOT = 8              # query slots per core
EC = D // 128          # 8 feature chunks
F32 = mybir.dt.float32
BF16 = mybir.dt.bfloat16
NEG = -1.0e9

_compiled = None


def _slot_T(s):
    return 256 * (s + 1)


def _build():
    nc = bacc.Bacc("TRN2", target_bir_lowering=False, debug=False, num_devices=8)

    xqT = nc.dram_tensor("xqT", [128, EC, 1024], BF16, kind="ExternalInput")
    xkvT = nc.dram_tensor("xkvT", [128, EC, S], BF16, kind="ExternalInput")
    wqT = nc.dram_tensor("wqT", [128, EC, D], BF16, kind="ExternalInput")
    wkT = nc.dram_tensor("wkT", [128, EC, D], BF16, kind="ExternalInput")
    wvT = nc.dram_tensor("wvT", [128, EC, D], BF16, kind="ExternalInput")
    woT = nc.dram_tensor("woT", [128, EC, D], BF16, kind="ExternalInput")
    bq_d = nc.dram_tensor("bq", [128, EC], F32, kind="ExternalInput")
    bk_d = nc.dram_tensor("bk", [128, EC], F32, kind="ExternalInput")
    bv_d = nc.dram_tensor("bv", [1, D], F32, kind="ExternalInput")
    bo_d = nc.dram_tensor("bo", [1, D], F32, kind="ExternalInput")
    mask_d = nc.dram_tensor("mask", [128, NSLOT, 256], F32, kind="ExternalInput")
    out_d = nc.dram_tensor("out", [NSLOT * 128, D], F32, kind="ExternalOutput")

    inv = 1.0 / 32.0

    with TileContext(nc) as tc:
        with (
            tc.tile_pool(name="persist", bufs=1) as persist,
            tc.tile_pool(name="small", bufs=1) as small,
        ):
            QT = persist.tile([128, EC, 1024], BF16, tag="QT")
            KT = persist.tile([128, EC, S], BF16, tag="KT")
            V = persist.tile([128, NB, D], BF16, tag="V")
            MASK = small.tile([128, NSLOT, 256], F32, tag="MASK")
            BQ = small.tile([128, EC], F32, tag="BQ")
            BK = small.tile([128, EC], F32, tag="BK")
            RL = small.tile([128, NSLOT], F32, tag="RL")
            BOF = small.tile([128, D], F32, tag="BOF")

            # ---- phase A: projections ----
            with (
                tc.tile_pool(name="xin", bufs=1) as xin,
                tc.tile_pool(name="wts", bufs=1) as wts,
                tc.tile_pool(name="pa_psum", bufs=4, space="PSUM") as pa_psum,
            ):
                XQ = xin.tile([128, EC, 1024], BF16, tag="XQ")
                XK = xin.tile([128, EC, S], BF16, tag="XK")
                WQ = wts.tile([128, EC, D], BF16, tag="WQ")
                WK = wts.tile([128, EC, D], BF16, tag="WK")
                WV = wts.tile([128, EC, D], BF16, tag="WV")
                BVF = xin.tile([128, D], F32, tag="BVF")

                # chunked loads in compute-need order: Q needs (XQ, WQ) first
                for dc in range(EC):
                    nc.sync.dma_start(out=XQ[:, dc, :], in_=xqT[:, dc, :])
                    nc.sync.dma_start(out=WQ[:, dc, :], in_=wqT[:, dc, :])
                bq_raw = small.tile([128, EC], F32, tag="bq_raw")
                nc.sync.dma_start(out=bq_raw[:, :], in_=bq_d[:, :])
                nc.scalar.mul(BQ[:, :], bq_raw[:, :], inv)
                for dc in range(EC):
                    nc.sync.dma_start(out=XK[:, dc, :], in_=xkvT[:, dc, :])
                    nc.sync.dma_start(out=WK[:, dc, :], in_=wkT[:, dc, :])
                nc.sync.dma_start(out=BK[:, :], in_=bk_d[:, :])
                for dc in range(EC):
                    nc.sync.dma_start(out=WV[:, dc, :], in_=wvT[:, dc, :])
                bv_row = small.tile([1, D], F32, tag="bv_row")
                nc.sync.dma_start(out=bv_row[:, :], in_=bv_d[:, :])
                nc.gpsimd.partition_broadcast(BVF[:, :], bv_row[:1, :])
                nc.sync.dma_start(out=MASK[:, :, :], in_=mask_d[:, :, :])
                bo_row = small.tile([1, D], F32, tag="bo_row")
                nc.sync.dma_start(out=bo_row[:, :], in_=bo_d[:, :])
                nc.gpsimd.partition_broadcast(BOF[:, :], bo_row[:1, :])

                # QT[e, s] (pre-scaled by 1/32): lhsT = WQ[d,e], rhs = XQ[d,s]
                for ec in range(EC):
                    for sh in range(2):
                        ps = pa_psum.tile([128, 512], F32, tag="pa")
                        for dc in range(EC):
                            nc.tensor.matmul(
                                ps[:, :],
                                WQ[:, dc, ec * 128 : (ec + 1) * 128],
                                XQ[:, dc, sh * 512 : (sh + 1) * 512],
                                start=(dc == 0),
                                stop=(dc == EC - 1),
                            )
                        # DVE: out = psum * 1/32 + bq/32  (bf16 on write)
                        nc.vector.tensor_scalar(
                            out=QT[:, ec, sh * 512 : (sh + 1) * 512],
                            in0=ps[:, :],
                            scalar1=inv,
                            scalar2=BQ[:, ec : ec + 1],
                            op0=mybir.AluOpType.mult,
                            op1=mybir.AluOpType.add,
                        )

                # KT[e, t]: lhsT = WK[d,e], rhs = XK[d,t]
                for ec in range(EC):
                    for th in range(S // 512):
                        ps = pa_psum.tile([128, 512], F32, tag="pa")
                        for dc in range(EC):
                            nc.tensor.matmul(
                                ps[:, :],
                                WK[:, dc, ec * 128 : (ec + 1) * 128],
                                XK[:, dc, th * 512 : (th + 1) * 512],
                                start=(dc == 0),
                                stop=(dc == EC - 1),
                            )
                        nc.vector.tensor_scalar(
                            out=KT[:, ec, th * 512 : (th + 1) * 512],
                            in0=ps[:, :],
                            scalar1=BK[:, ec : ec + 1],
                            scalar2=None,
                            op0=mybir.AluOpType.add,
                        )

                # V[t, d] natural: lhsT = XK[d, t-block], rhs = WV[d, e]
                for tb in range(NB):
                    for dh in range(2):
                        ps = pa_psum.tile([128, 512], F32, tag="pa")
                        for dc in range(EC):
                            nc.tensor.matmul(
                                ps[:, :],
                                XK[:, dc, tb * 128 : (tb + 1) * 128],
                                WV[:, dc, dh * 512 : (dh + 1) * 512],
                                start=(dc == 0),
                                stop=(dc == EC - 1),
                            )
                        nc.vector.tensor_tensor(
                            out=V[:, tb, dh * 512 : (dh + 1) * 512],
                            in0=ps[:, :],
                            in1=BVF[:, dh * 512 : (dh + 1) * 512],
                            op=mybir.AluOpType.add,
                        )

            # ---- phase B + C: attention + output projection ----
            with (
                tc.tile_pool(name="wo", bufs=1) as wo_pool,
                tc.tile_pool(name="att", bufs=5) as att_pool,
                tc.tile_pool(name="attT", bufs=2) as attT_pool,
                tc.tile_pool(name="ctx", bufs=1) as ctx_pool,
                tc.tile_pool(name="stat", bufs=1) as stat_pool,
                tc.tile_pool(name="sc_psum", bufs=2, space="PSUM") as sc_psum,
                tc.tile_pool(name="av_psum", bufs=2, space="PSUM") as av_psum,
                tc.tile_pool(name="out_psum", bufs=2, space="PSUM") as out_psum,
                tc.tile_pool(name="outbuf", bufs=2) as out_pool,
            ):
                WO = wo_pool.tile([128, EC, D], BF16, tag="WO")
                for dc in range(EC):
                    nc.sync.dma_start(out=WO[:, dc, :], in_=woT[:, dc, :])
                CTXT = ctx_pool.tile([128, EC, 1024], BF16, tag="CTXT")
                NM = stat_pool.tile([128, 2], F32, tag="NM")
                LSUM = stat_pool.tile([128, 2], F32, tag="LS")
                LTOT = stat_pool.tile([128, 1], F32, tag="LT")

                def out_proj(slot):
                    OUTS = out_pool.tile([128, D], F32, tag="outs")
                    for eh in range(2):
                        ps = out_psum.tile([128, 512], F32, tag="op")
                        for dc in range(EC):
                            nc.tensor.matmul(
                                ps[:, :],
                                CTXT[:, dc, slot * 128 : (slot + 1) * 128],
                                WO[:, dc, eh * 512 : (eh + 1) * 512],
                                start=(dc == 0),
                                stop=(dc == EC - 1),
                            )
                        nc.vector.scalar_tensor_tensor(
                            out=OUTS[:, eh * 512 : (eh + 1) * 512],
                            in0=ps[:, :],
                            scalar=RL[:, slot : slot + 1],
                            in1=BOF[:, eh * 512 : (eh + 1) * 512],
                            op0=mybir.AluOpType.mult,
                            op1=mybir.AluOpType.add,
                        )
                    nc.sync.dma_start(
                        out=out_d[slot * 128 : (slot + 1) * 128, :], in_=OUTS[:, :]
                    )

                for g in range(2):  # two groups of 4 slots
                    ATT_T = attT_pool.tile([128, NB, 512], BF16, tag="attT")
                    for j in range(4):
                        slot = g * 4 + j
                        T = _slot_T(slot)
                        nt = T // 128
                        ATT = att_pool.tile([128, S], BF16, tag="att")

                        nparts = (T + 1023) // 1024
                        parts = []
                        for p in range(nparts):
                            w = min(1024, T - p * 1024)
                            sc = sc_psum.tile([128, 1024], F32, tag="sc")
                            parts.append((sc, w))
                            for c0 in range(0, w, 512):
                                cw = min(512, w - c0)
                                for ec in range(EC):
                                    nc.tensor.matmul(
                                        sc[:, c0 : c0 + cw],
                                        QT[:, ec, slot * 128 : (slot + 1) * 128],
                                        KT[:, ec, p * 1024 + c0 : p * 1024 + c0 + cw],
                                        start=(ec == 0),
                                        stop=(ec == EC - 1),
                                    )
                        # additive causal mask on the last 256 cols
                        lsc, lw = parts[-1]
                        nc.vector.tensor_tensor(
                            out=lsc[:, lw - 256 : lw],
                            in0=lsc[:, lw - 256 : lw],
                            in1=MASK[:, slot, :],
                            op=mybir.AluOpType.add,
                        )
                        for p, (sc, w) in enumerate(parts):
                            nc.vector.reduce_max(
                                out=NM[:, p : p + 1],
                                in_=sc[:, :w],
                                axis=mybir.AxisListType.X,
                                negate=True,
                            )
                        if nparts == 2:
                            nc.vector.tensor_tensor(
                                out=NM[:, 0:1],
                                in0=NM[:, 0:1],
                                in1=NM[:, 1:2],
                                op=mybir.AluOpType.min,
                            )
                        for p, (sc, w) in enumerate(parts):
                            nc.scalar.activation(
                                ATT[:, p * 1024 : p * 1024 + w],
                                sc[:, :w],
                                mybir.ActivationFunctionType.Exp,
                                bias=NM[:, 0:1],
                                scale=1.0,
                                accum_out=LSUM[:, p : p + 1],
                            )
                        if nparts == 2:
                            nc.vector.tensor_tensor(
                                out=LTOT[:, 0:1],
                                in0=LSUM[:, 0:1],
                                in1=LSUM[:, 1:2],
                                op=mybir.AluOpType.add,
                            )
                            nc.vector.reciprocal(RL[:, slot : slot + 1], LTOT[:, 0:1])
                        else:
                            nc.vector.reciprocal(RL[:, slot : slot + 1], LSUM[:, 0:1])

                        # one batched xbar transpose per slot:
                        # [128 s, T] -> strip [128 t-part, nt, 128 s]
                        nc.sync.dma_start_transpose(
                            ATT_T[:, 0:nt, j * 128 : (j + 1) * 128],
                            ATT[:, 0:T],
                        )

                    # group AV: ctxT[d, s] accumulated over t chunks
                    ntg = _slot_T(g * 4 + 3) // 128  # 8 or 16
                    for dc in range(EC):
                        ps = av_psum.tile([128, 512], F32, tag="av")
                        for tcn in range(ntg):
                            jmin = 0
                            for jj in range(4):
                                if 256 * (g * 4 + jj + 1) >= 128 * (tcn + 1):
                                    jmin = jj
                                    break
                            scol = jmin * 128
                            nc.tensor.matmul(
                                ps[:, scol:512],
                                V[:, tcn, dc * 128 : (dc + 1) * 128],
                                ATT_T[:, tcn, scol:512],
                                start=(tcn == 0),
                                stop=(tcn == ntg - 1),
                            )
                        nc.vector.tensor_copy(
                            CTXT[:, dc, g * 512 : (g + 1) * 512], ps[:, :]
                        )
                    # emit out-proj for this group's slots right away so the
                    # scheduler can fill PE gaps during the next group's
                    # softmax/transpose latency
                    for j in range(4):
                        out_proj(g * 4 + j)

    nc.compile()
    return nc


def _core_blocks(core):
    parity = 1 if core % 2 == 0 else 0  # even core -> odd blocks
    return [2 * s + parity for s in range(NSLOT)]


def _make_in_maps(x, Wq, bq, Wk, bk, Wv, bv, Wo, bo):
    bf = ml_dtypes.bfloat16

    def wt_layout(W):
        return np.ascontiguousarray(
            W.T.astype(bf).reshape(EC, 128, D).transpose(1, 0, 2)
        )

    wq_l, wk_l, wv_l, wo_l = (wt_layout(W) for W in (Wq, Wk, Wv, Wo))
    bq_l = np.ascontiguousarray(bq.reshape(EC, 128).T.astype(np.float32))
    bk_l = np.ascontiguousarray(bk.reshape(EC, 128).T.astype(np.float32))
    bv_l = np.ascontiguousarray(bv.reshape(1, D).astype(np.float32))
    bo_l = np.ascontiguousarray(bo.reshape(1, D).astype(np.float32))

    in_maps = []
    for core in range(8):
        b = core // 2
        blocks = _core_blocks(core)
        xb = np.asarray(x[b], dtype=np.float32)
        xq = np.concatenate([xb[bl * 128 : (bl + 1) * 128] for bl in blocks], axis=0)
        xqT_l = np.ascontiguousarray(
            xq.T.astype(bf).reshape(EC, 128, 1024).transpose(1, 0, 2)
        )
        xkvT_l = np.ascontiguousarray(
            xb.T.astype(bf).reshape(EC, 128, S).transpose(1, 0, 2)
        )
        mask = np.zeros((128, NSLOT, 256), np.float32)
        r = np.arange(128)[:, None]
        jj = np.arange(256)[None, :]
        for s_i, bl in enumerate(blocks):
            lim = bl * 128 + r
            t_idx = 256 * s_i + jj
            mask[:, s_i, :] = np.where(t_idx <= lim, 0.0, NEG)
        in_maps.append(
            {
                "xqT": xqT_l,
                "xkvT": xkvT_l,
                "wqT": wq_l,
                "wkT": wk_l,
                "wvT": wv_l,
                "woT": wo_l,
                "bq": bq_l,
                "bk": bk_l,
                "bv": bv_l,
                "bo": bo_l,
                "mask": mask,
            }
        )
    return in_maps


def _run(inputs, trace=False):
    global _compiled
    if _compiled is None:
        _compiled = _build()
    nc = _compiled
    in_maps = _make_in_maps(**inputs)
    res = run_bass_kernel_spmd(nc, in_maps, core_ids=list(range(8)), trace=trace)
    out = np.zeros((B, S, D), np.float32)
    for core in range(8):
        b = core // 2
        o = res.results[core]["out"]
        for s_i, bl in enumerate(_core_blocks(core)):
            out[b, bl * 128 : (bl + 1) * 128, :] = o[s_i * 128 : (s_i + 1) * 128, :]
    return out, res


def kernel(**inputs):
    out, _ = _run(inputs, trace=False)
    return out
